# revision 52
# baseline (speedup 1.0000x reference)
"""BFP-quantized 3x3 conv (nn_BFConv2d) on 8 TRN2 NeuronCores.

Strategy (data-parallel over batch, 4 samples/core, ONE fused program):
  The reference BFP-quantizes x and w to 8 mantissa bits at a shared group
  exponent, then convolves. A round-to-nearest bf16 representation of x
  keeps 8 mantissa bits per element (finer than the reference lattice
  except for the group-max element); measured end-to-end error vs the
  exact reference conv is ~5e-3 relative, well inside the 2e-2 gate. The
  weight (37K elems) is exact-BFP-quantized host-side and cast to bf16.

  Host marshals x to bf16 with rows at stride 113 (112 px + ONE trailing
  zero: the shared slot is right-pad of row r AND left-pad of row r+1)
  so the device DMAs slabs straight into padded conv tiles with one
  strided run per partition - no staging pass, no on-device cast, and
  each 4-row matmul streams 451 columns instead of 456.

  Measured structure (perfetto): exec time = last activity minus first
  ENGINE instruction (~6.0us, pinned by framework ring-init memsets), and
  ends ~8.8us of fixed Tile semaphore-epilogue after the last DMA. The
  matmul phase (~50us) sits at the bf16 PE roofline: 9 taps x 4-quadrant
  64x64 tile_position packing = full 128x128 MAC utilization; fp8 fails
  the accuracy gate and Winograd loses on the 1x-mode PSUM-read tax, so
  the wins are all in the head/tail and in keeping the PE fed:

  Per core, per sample pair (A on SBUF partitions 0-63, B on 64-127):
    - 14 row-slabs of 8 rows, each in its own 10-row halo tile from a
      6-deep ring: the ring's WAR dependency demand-paces the input DMA
      (issuing all loads upfront floods the 16 shared SDMA engines at
      ~330GB/s, starves the output stores, and backpressure stalls the
      PE + re-throttles the HAM clock gate).
    - conv: per slab, 9 taps x 2 psum-row-halves x 2 samples = 36
      matmuls as 64x64 array tiles -> 4 concurrent = full PE, ~197ns
      per 4-matmul group over 456 columns. PSUM: 2 banks/slab, 4-deep.
    - warmup: vector-engine memset feeds 6-7 full-array dummy matmuls
      from ~7.7us so the HAM clock gate is at K=8/8 near the first real
      matmul (~10.5us, gated by slab-0's load); any PE-idle gap there
      can reset the HAM window and cost ~2-4us at 1.2GHz.
    - PSUM evacuation (strips width padding, fuses bias, emits bf16)
      split ScalarE/VectorE; 2 output DMAs per slab (one per 4-row
      block) on scalar HWDGE / gpsimd SWDGE; the last three slabs swap
      SWDGE for the then-idle sync ring (SWDGE's ~4us descriptor-to-
      data latency otherwise serializes into the kernel tail). Mid-
      kernel stores never ride the sync ring: their evac-waits would
      block the paced loads queued behind them (FIFO per ring).
  Output is written bf16 and cast to f32 on host (~2^-9 extra rounding).
"""

import os
import sys
from contextlib import ExitStack

import numpy as np

sys.path.insert(0, "/opt/trn_rl_repo")

import ml_dtypes  # noqa: E402
import concourse.bacc as bacc  # noqa: E402
import concourse.mybir as mybir  # noqa: E402
import concourse.tile as tile  # noqa: E402

F32 = mybir.dt.float32
BF16 = mybir.dt.bfloat16

N_CORES = 8
C = 64                      # channels (in == out)
H = W = 112
WP = W + 1                  # row stride 113: ONE shared zero between rows
                            # (right-pad of row r == left-pad of row r+1),
                            # so each 4-row matmul streams 451 cols not 456
SLAB = 8                    # output rows per pipeline slab
NSLAB = H // SLAB           # 14
TROWS = SLAB + 2            # slab tile rows incl halo
TCOLS = 1 + TROWS * WP      # left guard + 10 rows (each incl its trailing
                            # zero; the last row's trailing zero is col 1130)
SLEN = 3 * WP + W           # 451 matmul stream columns for 4 output rows
XBUFS = 6                   # slab-tile ring depth (paces input DMA;
                            # 3 proved too shallow - mid-stream PE gaps
                            # from DMA latency jitter + HAM oscillation)
OBUFS = 6                   # output staging ring depth (absorbs store
                            # backpressure when input DMA crowds the queues)
GROUP_MANTISSA = 8
GROUP_SIZE = 36

_cache = {}
last_exec_ns = {}
last_results = {}


def _trace_enabled():
    return os.environ.get("BFP_TRACE") == "1"


def _install_trace_shim():
    """Provide antenv.axon_hooks (NTFF profiling hook) if the image lacks it."""
    import types
    import ctypes
    import contextlib
    try:
        from antenv.axon_hooks import get_axon_ntff_profile_hook  # noqa: F401
        return
    except ImportError:
        pass
    so_path = "/opt/axon/libaxon_pjrt.so"
    if not os.path.exists(so_path):
        return
    lib = ctypes.CDLL(so_path)
    if not hasattr(lib, "axon_start_nrt_profile"):
        return
    lib.axon_start_nrt_profile.argtypes = [ctypes.POINTER(ctypes.c_int64),
                                           ctypes.c_size_t]
    lib.axon_start_nrt_profile.restype = ctypes.c_int64
    lib.axon_stop_nrt_profile.argtypes = [ctypes.c_char_p]
    lib.axon_stop_nrt_profile.restype = ctypes.c_int64

    @contextlib.contextmanager
    def _hook(output_dir, device_ids):
        import jax
        jax.devices()
        if device_ids:
            ids = (ctypes.c_int64 * len(device_ids))(*device_ids)
            rc = lib.axon_start_nrt_profile(ids, len(device_ids))
        else:
            rc = lib.axon_start_nrt_profile(None, 0)
        if rc != 0:
            raise RuntimeError(f"axon_start_nrt_profile rc={rc}")
        try:
            yield
        finally:
            n = lib.axon_stop_nrt_profile(str(output_dir).encode())
            print(f"profile: {n} ntff file(s) -> {output_dir}", file=sys.stderr)

    mod = types.ModuleType("antenv.axon_hooks")
    state = {"hook": _hook}
    mod.get_axon_ntff_profile_hook = lambda: state["hook"]
    mod.set_axon_ntff_profile_hook = lambda h: state.update(hook=h)
    sys.modules["antenv.axon_hooks"] = mod
    import antenv
    antenv.axon_hooks = mod
    from concourse import bass_utils as bu
    bu.upload_artifacts = lambda d: str(d)  # no egress from this container


def bfp_quantize_host(x, mantissa=GROUP_MANTISSA, group_size=GROUP_SIZE):
    """Exact reference BFP quantization (numpy, f64 intermediates)."""
    shape = np.asarray(x).shape
    flat = np.asarray(x, np.float32).reshape(-1).astype(np.float64)
    n = flat.shape[0]
    pad = (-n) % group_size
    f = np.pad(flat, (0, pad)).reshape(-1, group_size)
    m = np.max(np.abs(f), axis=1, keepdims=True)
    safe_m = np.where(m > 0, m, 1.0)
    e = np.floor(np.log2(safe_m))
    scale = np.exp2(e - (mantissa - 1))
    q = np.round(f / scale) * scale
    q = np.where(m > 0, q, 0.0)
    return q.reshape(-1)[:n].reshape(shape).astype(np.float32)


def build_fused():
    nc = bacc.Bacc(None)
    xin = nc.declare_dram_parameter("x", [4, C, H * WP], BF16, isOutput=False)
    wsb_d = nc.declare_dram_parameter("wsb", [128, 9 * 64], BF16, isOutput=False)
    bias_d = nc.declare_dram_parameter("bias2", [128], F32, isOutput=False)
    out = nc.declare_dram_parameter("out", [4, C, H, W], BF16, isOutput=True)

    with tile.TileContext(nc) as tc:
        with ExitStack() as ctx:
            consts = ctx.enter_context(tc.tile_pool(name="consts", bufs=1))
            xbpool = ctx.enter_context(tc.tile_pool(name="xb", bufs=XBUFS))
            opool = ctx.enter_context(tc.tile_pool(name="o", bufs=OBUFS))
            psum = ctx.enter_context(tc.tile_pool(name="ps", bufs=4, space="PSUM"))

            # HAM warmup: dummy full-array matmuls on a memset tile keep
            # the PE busy from ~6.6us so the clock gate is fully open
            # (K=8/8) by the first real matmul (~10.9us) - any PE-idle gap
            # in between can reset the HAM activity window and leave the
            # array at 1.2GHz for several microseconds. The memset rides
            # the otherwise-idle VECTOR engine (gpsimd's sequencer stalls
            # ~0.7us on the DMA-ring handshake; scalar is busy with
            # ACT_TABLE_LOAD). The exec clock start is pinned at ~6.0us by
            # framework ring-init memsets regardless, so early PE work
            # costs nothing. high_priority pins these first.
            wim = consts.tile([128, 512], BF16)
            wps = psum.tile([128, 512], F32, tag="psA")
            with tc.high_priority():
                # split memset: the first (short) warmup only needs cols
                # 0:128, so it can start while the rest is still zeroing
                nc.vector.memset(wim[:, 0:128], 0.0)
                nc.vector.memset(wim[:, 128:512], 0.0)
                nc.tensor.matmul(wps[:, 0:128], wim[:, 0:128],
                                 wim[:, 0:128], start=True, stop=True)
                for _ in range(5):
                    nc.tensor.matmul(wps[:, 0:512], wim[:, 0:128],
                                     wim[:, 0:512], start=True, stop=True)
                # short tail warmups close the gap to the first real
                # matmul: a PE-idle gap >~350ns there restarts the HAM
                # busy-window and delays full clock by up to 3.4us, so
                # cover the slab-0 arrival jitter (10.3-10.9us) entirely
                for _ in range(2):
                    nc.tensor.matmul(wps[:, 0:128], wim[:, 0:128],
                                     wim[:, 0:128], start=True, stop=True)

            # consts ride the scalar ring (idle until the first output
            # trigger) so slab-0's load leads the sync ring
            wsb = consts.tile([128, 9 * 64], BF16)
            nc.scalar.dma_start(wsb[:], wsb_d[:])
            bias_sb = consts.tile([128, 1], F32)
            nc.scalar.dma_start(bias_sb[:], bias_d[:, None])

            for p in range(2):
                for s in range(NSLAB):
                    r0 = SLAB * s
                    k = p * NSLAB + s
                    # Per-slab input tile: 10 padded rows (8 + 1-row halo
                    # each side) with zero guard cols. The XBUFS-deep ring
                    # makes load k wait (WAR) on slab k-XBUFS's matmuls, so
                    # input DMA is demand-paced instead of flooding the
                    # SDMA engines and starving the output stores.
                    xt = xbpool.tile([128, TCOLS], BF16, tag="xt")
                    if k < XBUFS and s != 0:
                        # left guard col, written once per physical buffer;
                        # later loads only overwrite the row region (each
                        # loaded row carries its own trailing zero)
                        nc.gpsimd.memset(xt[:, 0:1], 0.0)
                    tr_lo, tr_hi = 0, TROWS      # tile rows to load
                    if s == 0:
                        # zero top pad row incl guard col 0 and its
                        # trailing pad
                        nc.gpsimd.memset(xt[:, 0:1 + WP], 0.0)
                        tr_lo = 1
                    if s == NSLAB - 1:
                        # zero bottom pad row incl its trailing pad
                        nc.gpsimd.memset(
                            xt[:, 1 + (TROWS - 1) * WP:TCOLS], 0.0)
                        tr_hi = TROWS - 1
                    # tile row tr holds padded row r0+tr = input row r0+tr-1
                    in_lo = r0 + tr_lo - 1
                    in_hi = r0 + tr_hi - 1
                    if k == 0:
                        # split the critical first load: taps dh=0,1 (the
                        # first 6 matmul groups) only read tile rows 0-8,
                        # so they can start ~0.3us before row 9 lands
                        # (single_packet=True measured NO speedup here and
                        # is not worth the packetization risk)
                        nc.sync.dma_start(
                            xt[:, 1 + WP * tr_lo:1 + WP * (tr_hi - 1)],
                            xin[0:2, :, in_lo * WP:(in_hi - 1) * WP])
                        nc.sync.dma_start(
                            xt[:, 1 + WP * (tr_hi - 1):1 + WP * tr_hi],
                            xin[0:2, :, (in_hi - 1) * WP:in_hi * WP])
                    else:
                        nc.sync.dma_start(
                            xt[:, 1 + WP * tr_lo:1 + WP * tr_hi],
                            xin[2 * p:2 * p + 2, :, in_lo * WP:in_hi * WP])

                    # PSUM layout: bank pst[sm] holds sample sm, with
                    # partitions 0-63 = slab rows r0..r0+3 (cq=0) and
                    # partitions 64-127 = rows r0+4..r0+7 (cq=64).
                    pst = [psum.tile([128, 512], F32, tag=f"ps{'AB'[i]}",
                                     name=f"pst{i}")
                           for i in range(2)]
                    for t in range(9):
                        dh, dw = divmod(t, 3)
                        for sm in range(2):
                            for cq in (0, 64):
                                base = (dh + 4 * (cq // 64)) * WP + dw
                                nc.tensor.matmul(
                                    pst[sm][cq:cq + 64, 0:SLEN],
                                    wsb[64 * sm:64 * sm + 64,
                                        64 * t:64 * t + 64],
                                    xt[64 * sm:64 * sm + 64,
                                       base:base + SLEN],
                                    start=(t == 0), stop=(t == 8),
                                    tile_position=(64 * sm, cq))
                    # osb col layout per partition: (sm:2, rr:4, w:112);
                    # partition 64*q+oc holds rows r0+4q..r0+4q+3 of sample
                    # sm, channel oc, contiguous within each sm half.
                    osb = opool.tile([128, 2 * 4 * W], BF16, tag="osb")
                    for sm in range(2):
                        o0 = 448 * sm
                        edst = (osb[:, o0:o0 + 448]
                                .rearrange("p (r c) -> p r c", c=W))
                        esrc = (pst[sm][:, 0:4 * WP]
                                .rearrange("p (r c) -> p r c", c=WP)
                                [:, :, 0:W])
                        if sm == 0:
                            nc.scalar.activation(
                                edst, esrc,
                                mybir.ActivationFunctionType.Identity,
                                bias=bias_sb[:, 0:1], scale=1.0)
                        else:
                            nc.vector.tensor_scalar(
                                edst, esrc, bias_sb[:, 0:1], None,
                                op0=mybir.AluOpType.add)
                    # one DMA per 4-row block q covering both samples:
                    # DRAM AP dims (c, sm, rw) pair with src (part, sm, rw).
                    # Late slabs ride the sync ring (idle once loads finish)
                    # instead of the gpsimd SWDGE ring, whose ~4us
                    # descriptor-to-data latency otherwise blocks the tail.
                    ofl2 = (out[2 * p:2 * p + 2]
                            .rearrange("s c h w -> c s (h w)"))
                    late = (p == 1 and s >= NSLAB - 3)
                    for q in range(2):
                        c0 = (r0 + 4 * q) * W
                        if late:
                            deng = nc.sync if q == 1 else nc.scalar
                        else:
                            deng = nc.scalar if q == 0 else nc.gpsimd
                        deng.dma_start(
                            ofl2[:, :, c0:c0 + 4 * W],
                            osb[64 * q:64 * q + 64, :]
                            .rearrange("p (sm rw) -> p sm rw", sm=2))
    nc.compile()
    return nc


def _prep_weights(weight, bias):
    wq = bfp_quantize_host(np.asarray(weight, np.float32))   # [o, i, 3, 3]
    wtio = np.ascontiguousarray(wq.transpose(1, 2, 3, 0))    # [i, dh, dw, o]
    wsb = wtio.reshape(C, 9 * C)
    wsb = np.concatenate([wsb, wsb], axis=0).astype(ml_dtypes.bfloat16)
    bias2 = np.concatenate([np.asarray(bias, np.float32)] * 2)
    return wsb, bias2


def kernel(x, weight, bias):
    from concourse.bass_utils import run_bass_kernel_spmd

    if "fused" not in _cache:
        _cache["fused"] = build_fused()

    core_ids = list(range(N_CORES))
    trace = _trace_enabled()
    if trace:
        _install_trace_shim()

    wsb, bias2 = _prep_weights(weight, bias)
    xb16 = np.asarray(x, np.float32).astype(ml_dtypes.bfloat16)
    # each row is 112 px + ONE trailing zero (serves as right-pad of row r
    # AND left-pad of row r+1 in the 113-strided linear layout)
    xpadded = np.zeros((32, C, H, WP), ml_dtypes.bfloat16)
    xpadded[:, :, :, 0:W] = xb16
    xr = xpadded.reshape(N_CORES, 4, C, H * WP)
    in_maps = [{"x": xr[k], "wsb": wsb, "bias2": bias2}
               for k in range(N_CORES)]
    res = run_bass_kernel_spmd(_cache["fused"], in_maps, core_ids, trace=trace)
    last_exec_ns["fused"] = res.exec_time_ns
    last_results["fused"] = res

    out = np.concatenate(
        [np.asarray(res.results[k]["out"]) for k in range(N_CORES)], axis=0)
    return out.astype(np.float32).reshape(32, C, H, W)



# revision 53
# speedup vs baseline: 1.0287x; 1.0287x over previous
"""BFP-quantized 3x3 conv (nn_BFConv2d) on 8 TRN2 NeuronCores.

Strategy (data-parallel over batch, 4 samples/core, ONE fused program):
  The reference BFP-quantizes x and w to 8 mantissa bits at a shared group
  exponent, then convolves. A round-to-nearest bf16 representation of x
  keeps 8 mantissa bits per element (finer than the reference lattice
  except for the group-max element); measured end-to-end error vs the
  exact reference conv is ~5e-3 relative, well inside the 2e-2 gate. The
  weight (37K elems) is exact-BFP-quantized host-side and cast to bf16.

  Host marshals x to bf16 with rows at stride 113 (112 px + ONE trailing
  zero: the shared slot is right-pad of row r AND left-pad of row r+1)
  so the device DMAs slabs straight into padded conv tiles with one
  strided run per partition - no staging pass, no on-device cast, and
  each 4-row matmul streams 451 columns instead of 456.

  Measured structure (perfetto): exec time = last activity minus first
  ENGINE instruction (~6.0us, pinned by framework ring-init memsets), and
  ends ~8.8us of fixed Tile semaphore-epilogue after the last DMA. The
  matmul phase (~50us) sits at the bf16 PE roofline: 9 taps x 4-quadrant
  64x64 tile_position packing = full 128x128 MAC utilization; fp8 fails
  the accuracy gate and Winograd loses on the 1x-mode PSUM-read tax, so
  the wins are all in the head/tail and in keeping the PE fed:

  Per core, per sample pair (A on SBUF partitions 0-63, B on 64-127):
    - 14 row-slabs of 8 rows, each in its own 10-row halo tile from a
      6-deep ring: the ring's WAR dependency demand-paces the input DMA
      (issuing all loads upfront floods the 16 shared SDMA engines at
      ~330GB/s, starves the output stores, and backpressure stalls the
      PE + re-throttles the HAM clock gate).
    - conv: per slab, 9 taps x 2 psum-row-halves x 2 samples = 36
      matmuls as 64x64 array tiles -> 4 concurrent = full PE, ~197ns
      per 4-matmul group over 456 columns. PSUM: 2 banks/slab, 4-deep.
    - warmup: vector-engine memset feeds 6-7 full-array dummy matmuls
      from ~7.7us so the HAM clock gate is at K=8/8 near the first real
      matmul (~10.5us, gated by slab-0's load); any PE-idle gap there
      can reset the HAM window and cost ~2-4us at 1.2GHz.
    - PSUM evacuation (strips width padding, fuses bias, emits bf16)
      split ScalarE/VectorE; 2 output DMAs per slab (one per 4-row
      block) on scalar HWDGE / gpsimd SWDGE; the last three slabs swap
      SWDGE for the then-idle sync ring (SWDGE's ~4us descriptor-to-
      data latency otherwise serializes into the kernel tail). Mid-
      kernel stores never ride the sync ring: their evac-waits would
      block the paced loads queued behind them (FIFO per ring).
  Output is written bf16 and cast to f32 on host (~2^-9 extra rounding).
"""

import os
import sys
from contextlib import ExitStack

import numpy as np

sys.path.insert(0, "/opt/trn_rl_repo")

import ml_dtypes  # noqa: E402
import concourse.bacc as bacc  # noqa: E402
import concourse.mybir as mybir  # noqa: E402
import concourse.tile as tile  # noqa: E402

F32 = mybir.dt.float32
BF16 = mybir.dt.bfloat16

N_CORES = 8
C = 64                      # channels (in == out)
H = W = 112
WP = W + 1                  # row stride 113: ONE shared zero between rows
                            # (right-pad of row r == left-pad of row r+1),
                            # so each 4-row matmul streams 451 cols not 456
SLAB = 8                    # output rows per pipeline slab
NSLAB = H // SLAB           # 14
TROWS = SLAB + 2            # slab tile rows incl halo
TCOLS = 1 + TROWS * WP      # left guard + 10 rows (each incl its trailing
                            # zero; the last row's trailing zero is col 1130)
SLEN = 3 * WP + W           # 451 matmul stream columns for 4 output rows
XBUFS = 6                   # slab-tile ring depth (paces input DMA;
                            # 3 proved too shallow - mid-stream PE gaps
                            # from DMA latency jitter + HAM oscillation)
OBUFS = 6                   # output staging ring depth (absorbs store
                            # backpressure when input DMA crowds the queues)
GROUP_MANTISSA = 8
GROUP_SIZE = 36

_cache = {}
last_exec_ns = {}
last_results = {}


def _trace_enabled():
    return os.environ.get("BFP_TRACE") == "1"


def _install_trace_shim():
    """Provide antenv.axon_hooks (NTFF profiling hook) if the image lacks it."""
    import types
    import ctypes
    import contextlib
    try:
        from antenv.axon_hooks import get_axon_ntff_profile_hook  # noqa: F401
        return
    except ImportError:
        pass
    so_path = "/opt/axon/libaxon_pjrt.so"
    if not os.path.exists(so_path):
        return
    lib = ctypes.CDLL(so_path)
    if not hasattr(lib, "axon_start_nrt_profile"):
        return
    lib.axon_start_nrt_profile.argtypes = [ctypes.POINTER(ctypes.c_int64),
                                           ctypes.c_size_t]
    lib.axon_start_nrt_profile.restype = ctypes.c_int64
    lib.axon_stop_nrt_profile.argtypes = [ctypes.c_char_p]
    lib.axon_stop_nrt_profile.restype = ctypes.c_int64

    @contextlib.contextmanager
    def _hook(output_dir, device_ids):
        import jax
        jax.devices()
        if device_ids:
            ids = (ctypes.c_int64 * len(device_ids))(*device_ids)
            rc = lib.axon_start_nrt_profile(ids, len(device_ids))
        else:
            rc = lib.axon_start_nrt_profile(None, 0)
        if rc != 0:
            raise RuntimeError(f"axon_start_nrt_profile rc={rc}")
        try:
            yield
        finally:
            n = lib.axon_stop_nrt_profile(str(output_dir).encode())
            print(f"profile: {n} ntff file(s) -> {output_dir}", file=sys.stderr)

    mod = types.ModuleType("antenv.axon_hooks")
    state = {"hook": _hook}
    mod.get_axon_ntff_profile_hook = lambda: state["hook"]
    mod.set_axon_ntff_profile_hook = lambda h: state.update(hook=h)
    sys.modules["antenv.axon_hooks"] = mod
    import antenv
    antenv.axon_hooks = mod
    from concourse import bass_utils as bu
    bu.upload_artifacts = lambda d: str(d)  # no egress from this container


def bfp_quantize_host(x, mantissa=GROUP_MANTISSA, group_size=GROUP_SIZE):
    """Exact reference BFP quantization (numpy, f64 intermediates)."""
    shape = np.asarray(x).shape
    flat = np.asarray(x, np.float32).reshape(-1).astype(np.float64)
    n = flat.shape[0]
    pad = (-n) % group_size
    f = np.pad(flat, (0, pad)).reshape(-1, group_size)
    m = np.max(np.abs(f), axis=1, keepdims=True)
    safe_m = np.where(m > 0, m, 1.0)
    e = np.floor(np.log2(safe_m))
    scale = np.exp2(e - (mantissa - 1))
    q = np.round(f / scale) * scale
    q = np.where(m > 0, q, 0.0)
    return q.reshape(-1)[:n].reshape(shape).astype(np.float32)


def build_fused():
    nc = bacc.Bacc(None)
    xin = nc.declare_dram_parameter("x", [4, C, H * WP], BF16, isOutput=False)
    wsb_d = nc.declare_dram_parameter("wsb", [128, 9 * 64], BF16, isOutput=False)
    bias_d = nc.declare_dram_parameter("bias2", [128], F32, isOutput=False)
    out = nc.declare_dram_parameter("out", [4, C, H, W], BF16, isOutput=True)

    with tile.TileContext(nc) as tc:
        with ExitStack() as ctx:
            consts = ctx.enter_context(tc.tile_pool(name="consts", bufs=1))
            xbpool = ctx.enter_context(tc.tile_pool(name="xb", bufs=XBUFS))
            opool = ctx.enter_context(tc.tile_pool(name="o", bufs=OBUFS))
            psum = ctx.enter_context(tc.tile_pool(name="ps", bufs=4, space="PSUM"))

            # HAM warmup: dummy full-array matmuls on a memset tile keep
            # the PE busy from ~6.6us so the clock gate is fully open
            # (K=8/8) by the first real matmul (~10.9us) - any PE-idle gap
            # in between can reset the HAM activity window and leave the
            # array at 1.2GHz for several microseconds. The memset rides
            # the otherwise-idle VECTOR engine (gpsimd's sequencer stalls
            # ~0.7us on the DMA-ring handshake; scalar is busy with
            # ACT_TABLE_LOAD). The exec clock start is pinned at ~6.0us by
            # framework ring-init memsets regardless, so early PE work
            # costs nothing. high_priority pins these first.
            wim = consts.tile([128, 512], BF16)
            wps = psum.tile([128, 512], F32, tag="psA")
            with tc.high_priority():
                # split memset: the first (short) warmup only needs cols
                # 0:128, so it can start while the rest is still zeroing
                nc.vector.memset(wim[:, 0:128], 0.0)
                nc.vector.memset(wim[:, 128:512], 0.0)
                nc.tensor.matmul(wps[:, 0:128], wim[:, 0:128],
                                 wim[:, 0:128], start=True, stop=True)
                for _ in range(5):
                    nc.tensor.matmul(wps[:, 0:512], wim[:, 0:128],
                                     wim[:, 0:512], start=True, stop=True)
                # short tail warmup closes the gap to the first real
                # matmul (a PE-idle gap >~350ns there restarts the HAM
                # busy-window and delays full clock by up to 3.4us).
                # Exactly ONE short warmup: chaining two N=128 warmups
                # measurably leaves 200-500ns gaps BETWEEN them at cold
                # clock, which itself triggers the reset it should avoid.
                nc.tensor.matmul(wps[:, 0:128], wim[:, 0:128],
                                 wim[:, 0:128], start=True, stop=True)

            # consts ride the scalar ring (idle until the first output
            # trigger) so slab-0's load leads the sync ring
            wsb = consts.tile([128, 9 * 64], BF16)
            nc.scalar.dma_start(wsb[:], wsb_d[:])
            bias_sb = consts.tile([128, 1], F32)
            nc.scalar.dma_start(bias_sb[:], bias_d[:, None])

            for p in range(2):
                for s in range(NSLAB):
                    r0 = SLAB * s
                    k = p * NSLAB + s
                    # Per-slab input tile: 10 padded rows (8 + 1-row halo
                    # each side) with zero guard cols. The XBUFS-deep ring
                    # makes load k wait (WAR) on slab k-XBUFS's matmuls, so
                    # input DMA is demand-paced instead of flooding the
                    # SDMA engines and starving the output stores.
                    xt = xbpool.tile([128, TCOLS], BF16, tag="xt")
                    if k < XBUFS and s != 0:
                        # left guard col, written once per physical buffer;
                        # later loads only overwrite the row region (each
                        # loaded row carries its own trailing zero)
                        nc.gpsimd.memset(xt[:, 0:1], 0.0)
                    tr_lo, tr_hi = 0, TROWS      # tile rows to load
                    if s == 0:
                        # zero top pad row incl guard col 0 and its
                        # trailing pad
                        nc.gpsimd.memset(xt[:, 0:1 + WP], 0.0)
                        tr_lo = 1
                    if s == NSLAB - 1:
                        # zero bottom pad row incl its trailing pad
                        nc.gpsimd.memset(
                            xt[:, 1 + (TROWS - 1) * WP:TCOLS], 0.0)
                        tr_hi = TROWS - 1
                    # tile row tr holds padded row r0+tr = input row r0+tr-1
                    in_lo = r0 + tr_lo - 1
                    in_hi = r0 + tr_hi - 1
                    if k == 0:
                        # split the critical first load: taps dh=0,1 (the
                        # first 6 matmul groups) only read tile rows 0-8,
                        # so they can start ~0.3us before row 9 lands
                        # (single_packet=True measured NO speedup here and
                        # is not worth the packetization risk)
                        nc.sync.dma_start(
                            xt[:, 1 + WP * tr_lo:1 + WP * (tr_hi - 1)],
                            xin[0:2, :, in_lo * WP:(in_hi - 1) * WP])
                        nc.sync.dma_start(
                            xt[:, 1 + WP * (tr_hi - 1):1 + WP * tr_hi],
                            xin[0:2, :, (in_hi - 1) * WP:in_hi * WP])
                    else:
                        nc.sync.dma_start(
                            xt[:, 1 + WP * tr_lo:1 + WP * tr_hi],
                            xin[2 * p:2 * p + 2, :, in_lo * WP:in_hi * WP])

                    # PSUM layout: bank pst[sm] holds sample sm, with
                    # partitions 0-63 = slab rows r0..r0+3 (cq=0) and
                    # partitions 64-127 = rows r0+4..r0+7 (cq=64).
                    pst = [psum.tile([128, 512], F32, tag=f"ps{'AB'[i]}",
                                     name=f"pst{i}")
                           for i in range(2)]
                    for t in range(9):
                        dh, dw = divmod(t, 3)
                        for sm in range(2):
                            for cq in (0, 64):
                                base = (dh + 4 * (cq // 64)) * WP + dw
                                nc.tensor.matmul(
                                    pst[sm][cq:cq + 64, 0:SLEN],
                                    wsb[64 * sm:64 * sm + 64,
                                        64 * t:64 * t + 64],
                                    xt[64 * sm:64 * sm + 64,
                                       base:base + SLEN],
                                    start=(t == 0), stop=(t == 8),
                                    tile_position=(64 * sm, cq))
                    # osb col layout per partition: (sm:2, rr:4, w:112);
                    # partition 64*q+oc holds rows r0+4q..r0+4q+3 of sample
                    # sm, channel oc, contiguous within each sm half.
                    osb = opool.tile([128, 2 * 4 * W], BF16, tag="osb")
                    for sm in range(2):
                        o0 = 448 * sm
                        edst = (osb[:, o0:o0 + 448]
                                .rearrange("p (r c) -> p r c", c=W))
                        esrc = (pst[sm][:, 0:4 * WP]
                                .rearrange("p (r c) -> p r c", c=WP)
                                [:, :, 0:W])
                        if sm == 0:
                            nc.scalar.activation(
                                edst, esrc,
                                mybir.ActivationFunctionType.Identity,
                                bias=bias_sb[:, 0:1], scale=1.0)
                        else:
                            nc.vector.tensor_scalar(
                                edst, esrc, bias_sb[:, 0:1], None,
                                op0=mybir.AluOpType.add)
                    # one DMA per 4-row block q covering both samples:
                    # DRAM AP dims (c, sm, rw) pair with src (part, sm, rw).
                    # Late slabs ride the sync ring (idle once loads finish)
                    # instead of the gpsimd SWDGE ring, whose ~4us
                    # descriptor-to-data latency otherwise blocks the tail.
                    ofl2 = (out[2 * p:2 * p + 2]
                            .rearrange("s c h w -> c s (h w)"))
                    late = (p == 1 and s >= NSLAB - 3)
                    for q in range(2):
                        c0 = (r0 + 4 * q) * W
                        if late:
                            deng = nc.sync if q == 1 else nc.scalar
                        else:
                            deng = nc.scalar if q == 0 else nc.gpsimd
                        deng.dma_start(
                            ofl2[:, :, c0:c0 + 4 * W],
                            osb[64 * q:64 * q + 64, :]
                            .rearrange("p (sm rw) -> p sm rw", sm=2))
    nc.compile()
    return nc


def _prep_weights(weight, bias):
    wq = bfp_quantize_host(np.asarray(weight, np.float32))   # [o, i, 3, 3]
    wtio = np.ascontiguousarray(wq.transpose(1, 2, 3, 0))    # [i, dh, dw, o]
    wsb = wtio.reshape(C, 9 * C)
    wsb = np.concatenate([wsb, wsb], axis=0).astype(ml_dtypes.bfloat16)
    bias2 = np.concatenate([np.asarray(bias, np.float32)] * 2)
    return wsb, bias2


def kernel(x, weight, bias):
    from concourse.bass_utils import run_bass_kernel_spmd

    if "fused" not in _cache:
        _cache["fused"] = build_fused()

    core_ids = list(range(N_CORES))
    trace = _trace_enabled()
    if trace:
        _install_trace_shim()

    wsb, bias2 = _prep_weights(weight, bias)
    xb16 = np.asarray(x, np.float32).astype(ml_dtypes.bfloat16)
    # each row is 112 px + ONE trailing zero (serves as right-pad of row r
    # AND left-pad of row r+1 in the 113-strided linear layout)
    xpadded = np.zeros((32, C, H, WP), ml_dtypes.bfloat16)
    xpadded[:, :, :, 0:W] = xb16
    xr = xpadded.reshape(N_CORES, 4, C, H * WP)
    in_maps = [{"x": xr[k], "wsb": wsb, "bias2": bias2}
               for k in range(N_CORES)]
    res = run_bass_kernel_spmd(_cache["fused"], in_maps, core_ids, trace=trace)
    last_exec_ns["fused"] = res.exec_time_ns
    last_results["fused"] = res

    out = np.concatenate(
        [np.asarray(res.results[k]["out"]) for k in range(N_CORES)], axis=0)
    return out.astype(np.float32).reshape(32, C, H, W)

